# revision 2
# baseline (speedup 1.0000x reference)
"""Cross-attention scores kernel, v2: fp8 DoubleRow + exact top-16 correction.

Pass 1 (approximate, fp8):
    eo is streamed as fp8e4(32*eo) [1 B/elem => ~47us HBM floor/core], and the
    projection We@eo runs as DoubleRow fp8 matmuls (256-deep contraction per
    instruction) against fp8e4(64*We).  tanh(psum/2048 + base) on ACT -> bf16
    energies -> v-dot matmuls -> exp + running sums (as in the bf16 baseline).
    fp8 quantization gives ~7e-2 max rel err on the softmax output -- too big.

Selection (overlapped on idle DVE):
    After each s-group's exp lands in out_sb, max8 + max_index record the
    group's top-8 exp values and indices (per batch; batch bb lives on
    partition 32*(bb%4)+bb//4).

Correction tail:
    From the 64 candidates per batch pick the top 16 by value (max8 +
    match_replace rounds), extract their s-indices, gather those eo columns
    (bf16 rows of eoG) with one 128-row indirect DMA, recompute their scores
    exactly in bf16, and patch: Z' = Z - sum(exp approx@sel) + sum(exp exact@
    sel); output = approx exp * 1/Z' everywhere, then scatter the <=128 exact
    entries over it (same gpsimd DMA queue + explicit dep => ordered last).
    Simulated end-to-end on the real data: max rel err 6.3e-3 (vs 5.7e-3 for
    the all-bf16 baseline, 2e-2 budget).

Sharding: data-parallel over batch (64 -> 8 per core), weights replicated,
no collectives.
"""

import numpy as np
import ml_dtypes

import concourse.bass as bass
import concourse.bacc as bacc
import concourse.tile as tile
from concourse import mybir
from concourse.bass_utils import run_bass_kernel_spmd
from concourse.tile_rust import add_dep_helper
from concourse.masks import make_identity

dt = mybir.dt

S = 4096          # src_len
B = 64            # global batch
E2 = 512          # 2*enc_hid
D = 256           # dec_hid
NCORES = 8
BC = B // NCORES  # batches per core = 8
P = 128
SG = 512          # s-group size
NG = S // SG      # 8 s-groups
NEC = E2 // P     # 4 e-chunks
NKC = D // P      # 2 k-chunks
HB = 4            # batches per block-half
NB = NG * 2       # 16 blocks (s-group x batch-half)
NSEL = 16         # corrected entries per batch

SE = 32.0         # eo fp8 scale (|32*eo| < 240 = e4m3 max finite)
SW = 64.0         # We fp8 scale
INV_SCALE = 1.0 / (SE * SW)

F32 = dt.float32
BF16 = dt.bfloat16
FP8 = dt.float8e4
I32 = dt.int32
U16 = dt.uint16

DR = mybir.MatmulPerfMode.DoubleRow


def _p_of(bb):
    """Partition where batch bb's scores land (quadrant q=bb%4, col bb//4)."""
    return 32 * (bb % 4) + bb // 4


def _sel_col(bb):
    """Column base in the 128-wide selection axis for batch bb.

    Selection slot order follows the indirect-DMA offset AP iteration
    (q, r, j) with offset partitions pi(bb)=32q+r, so slot = 32q + 16r + j
    for bb = 4r + q.
    """
    q, r = bb % 4, bb // 4
    return 32 * q + 16 * r


def build_program():
    nc = bacc.Bacc(None, target_bir_lowering=False, debug=False, num_devices=8)

    # eoT8[bb, p, g, c, j] = fp8(SE * eo[g*512+j, bb, c*128+p])
    eoT8_d = nc.declare_dram_parameter("eoT8", [BC, P, NG, NEC, SG], FP8, isOutput=False)
    # eoG[bb*S + s, e] = bf16 eo[s, bb, e]  (gather rows for the exact pass)
    eoG_d = nc.declare_dram_parameter("eoG", [BC * S, E2], BF16, isOutput=False)
    # weT8[p, c, k] = fp8(SW * We.T[c*128+p, k])
    weT8_d = nc.declare_dram_parameter("weT8", [P, NEC, D], FP8, isOutput=False)
    # weTb[p, c, k] = bf16 We.T[c*128+p, k]   (exact pass)
    weTb_d = nc.declare_dram_parameter("weTb", [P, NEC, D], BF16, isOutput=False)
    # whT[p, dc, kc, j] = W[kc*128+j, dc*128+p]  (Wh part, f32)
    whT_d = nc.declare_dram_parameter("whT", [P, NKC, NKC, P], F32, isOutput=False)
    # hT[p, dc, bb] = h[bb, dc*128+p]
    hT_d = nc.declare_dram_parameter("hT", [P, NKC, BC], F32, isOutput=False)
    # bT[p, kc] = bias[kc*128+p]
    bT_d = nc.declare_dram_parameter("bT", [P, NKC], F32, isOutput=False)
    # vm[p, kc, bb, col] = v[kc*128+p] if col == bb//4 else 0  (bf16)
    vm_d = nc.declare_dram_parameter("vm", [P, NKC, BC, 32], BF16, isOutput=False)
    # vT[p, kc] = v[kc*128+p]  (bf16, exact-pass dot)
    vT_d = nc.declare_dram_parameter("vT", [P, NKC], BF16, isOutput=False)
    # bbofs[pi(bb)] = bb*S - 1 (host-built constant; gpsimd memset can't
    # target partition offsets)
    bbofs_d = nc.declare_dram_parameter("bbofs", [P, 1], F32, isOutput=False)
    # flat output: out[bb*S + s] = softmax scores
    out_d = nc.declare_dram_parameter("out", [BC * S, 1], F32, isOutput=True)

    with tile.TileContext(nc) as tc:
        with tc.tile_pool(name="consts", bufs=1) as consts:
            weT8 = consts.tile([P, NEC, D], FP8)
            nc.sync.dma_start(out=weT8, in_=weT8_d[:])
            whT = consts.tile([P, NKC, NKC, P], F32)
            nc.sync.dma_start(out=whT, in_=whT_d[:])
            hT = consts.tile([P, NKC, BC], F32)
            nc.sync.dma_start(out=hT, in_=hT_d[:])
            bT = consts.tile([P, NKC], F32)
            nc.sync.dma_start(out=bT, in_=bT_d[:])
            vm = consts.tile([P, NKC, BC, 32], BF16)
            nc.sync.dma_start(out=vm, in_=vm_d[:])
            weTb = consts.tile([P, NEC, D], BF16)
            nc.scalar.dma_start(out=weTb, in_=weTb_d[:])
            vT = consts.tile([P, NKC], BF16)
            nc.scalar.dma_start(out=vT, in_=vT_d[:])

            baseT = consts.tile([P, NKC, BC], F32)     # [k128, kc, bb]
            base_ex = consts.tile([P, NKC, P], F32)    # [k128, kc, selcol]
            esums = consts.tile([P, NG], F32)
            esum_run = consts.tile([P, 1], F32)
            nc.gpsimd.memset(esum_run, 0.0)
            out_sb = consts.tile([P, S], F32)          # exp(scores)

            gvals = consts.tile([P, NG, 8], F32)       # per-group top-8 exp
            gidxu = consts.tile([P, NG, 8], U16)       # their group-local idx
            gofs = consts.tile([P, NG, 8], F32)        # +g*512+1 offsets
            for g in range(NG):
                nc.gpsimd.memset(gofs[:, g, :], float(g * SG + 1))
            bbofs = consts.tile([P, 1], F32)           # bb*S - 1 on pi(bb)
            nc.scalar.dma_start(out=bbofs, in_=bbofs_d[:])
            ident = consts.tile([P, P], BF16)
            make_identity(nc, ident)
            # Zx preset so unused partitions stay finite through reciprocal
            Zx = consts.tile([P, 1], F32)
            nc.gpsimd.memset(Zx, 1.0)

            # --- init: baseT[k, bb] = sum_d Wh[k, d] h[bb, d] + bias[k] ---
            with tc.tile_pool(name="initps", bufs=1, space="PSUM") as initps:
                ps_base = initps.tile([P, NKC, BC], F32)
                for kc in range(NKC):
                    for dc in range(NKC):
                        nc.tensor.matmul(
                            ps_base[:, kc, :],
                            whT[:, dc, kc, :],
                            hT[:, dc, :],
                            start=(dc == 0),
                            stop=(dc == NKC - 1),
                        )
                for kc in range(NKC):
                    nc.vector.tensor_scalar_add(
                        baseT[:, kc, :], ps_base[:, kc, :], bT[:, kc : kc + 1]
                    )
            # expand baseT along the selection axis: base_ex[:, kc, col(bb)+j]
            for bb in range(BC):
                cs = _sel_col(bb)
                for kc in range(NKC):
                    nc.vector.tensor_copy(
                        base_ex[:, kc, cs : cs + NSEL],
                        baseT[:, kc, bb : bb + 1].to_broadcast([P, NSEL]),
                    )

            with (
                tc.tile_pool(name="eot", bufs=24) as eot_pool,
                tc.tile_pool(name="en", bufs=16) as en_pool,
                tc.tile_pool(name="pep", bufs=6, space="PSUM") as pep_pool,
                tc.tile_pool(name="psc", bufs=1, space="PSUM") as psc_pool,
            ):
                pend = None
                ps_sc = None

                def emit_dots(g, half, ens):
                    """v-dot matmuls for one block-half; 4 concurrent
                    quadrants; at half==1 exp + accum + candidate max8."""
                    nonlocal ps_sc
                    if half == 0:
                        ps_sc = psc_pool.tile([P, SG], F32, tag="psc")
                    for kc in range(NKC):
                        for i in range(HB):
                            bb = half * HB + i
                            q = bb % 4
                            nc.tensor.matmul(
                                ps_sc[32 * q : 32 * q + 32, :],
                                vm[:, kc, bb, :],
                                ens[(i, kc)],
                                start=(half == 0 and kc == 0),
                                stop=(half == 1 and kc == NKC - 1),
                                tile_position=(0, 32 * q),
                                # the sim's zero-region group check
                                # false-positives on multi-quadrant psum
                                # sharing (its value semantics are fine)
                                skip_group_check=True,
                            )
                    if half == 1:
                        s0 = g * SG
                        nc.scalar.activation(
                            out=out_sb[:, s0 : s0 + SG], in_=ps_sc,
                            func=mybir.ActivationFunctionType.Exp,
                            accum_out=esums[:, g : g + 1],
                        )
                        nc.vector.tensor_scalar_add(
                            esum_run, esum_run, esums[:, g : g + 1]
                        )
                        # group candidates for the correction pass
                        nc.vector.max(gvals[:, g, :], out_sb[:, s0 : s0 + SG])
                        nc.vector.max_index(
                            gidxu[:, g, :], gvals[:, g, :],
                            out_sb[:, s0 : s0 + SG],
                        )

                for b in range(NB):
                    g, half = b // 2, b % 2
                    eots = []
                    for i in range(HB):
                        bb = half * HB + i
                        t = eot_pool.tile([P, NEC, SG], FP8, tag="eot")
                        if b == 0:
                            # first block: split DMAs along the DoubleRow
                            # halves so matmul j=0 starts on the first half
                            nc.sync.dma_start(
                                out=t[:, :2, :], in_=eoT8_d[bb, :, g, :2, :]
                            )
                            nc.sync.dma_start(
                                out=t[:, 2:, :], in_=eoT8_d[bb, :, g, 2:, :]
                            )
                        else:
                            nc.sync.dma_start(out=t, in_=eoT8_d[bb, :, g])
                        eots.append(t)

                    ens = {}
                    if b == 0:
                        # batch-outer so the first matmul only waits on tile 0
                        for i in range(HB):
                            bb = half * HB + i
                            for kc in range(NKC):
                                ps = pep_pool.tile(
                                    [P, SG], F32, tag="pep", name=f"ps0_{kc}_{i}"
                                )
                                for j in range(2):
                                    nc.tensor.matmul(
                                        ps,
                                        weT8[:, 2 * j : 2 * j + 2,
                                             kc * P : (kc + 1) * P],
                                        eots[i][:, 2 * j : 2 * j + 2, :],
                                        start=(j == 0), stop=(j == 1),
                                        perf_mode=DR,
                                    )
                                en = en_pool.tile([P, SG], BF16, tag="en",
                                                  name=f"en0_{kc}_{i}")
                                nc.scalar.activation(
                                    out=en, in_=ps,
                                    func=mybir.ActivationFunctionType.Tanh,
                                    bias=baseT[:, kc, bb : bb + 1],
                                    scale=INV_SCALE,
                                )
                                ens[(i, kc)] = en
                        pend = (g, half, ens)
                        continue
                    for kc in range(NKC):
                        pss = [
                            pep_pool.tile([P, SG], F32, tag="pep",
                                          name=f"ps_{b}_{kc}_{i}")
                            for i in range(HB)
                        ]
                        for j in range(2):
                            lhsT = weT8[:, 2 * j : 2 * j + 2,
                                        kc * P : (kc + 1) * P]
                            for i in range(HB):
                                nc.tensor.matmul(
                                    pss[i], lhsT,
                                    eots[i][:, 2 * j : 2 * j + 2, :],
                                    start=(j == 0), stop=(j == 1),
                                    perf_mode=DR,
                                )
                        for i in range(HB):
                            bb = half * HB + i
                            en = en_pool.tile([P, SG], BF16, tag="en")
                            nc.scalar.activation(
                                out=en, in_=pss[i],
                                func=mybir.ActivationFunctionType.Tanh,
                                bias=baseT[:, kc, bb : bb + 1],
                                scale=INV_SCALE,
                            )
                            ens[(i, kc)] = en
                    if pend is not None:
                        emit_dots(*pend)
                    pend = (g, half, ens)

                emit_dots(*pend)

                # ================= correction tail =================
                with (
                    tc.tile_pool(name="tail", bufs=1) as tail,
                    tc.tile_pool(name="tps", bufs=1, space="PSUM") as tps,
                ):
                    gv = gvals[:].rearrange("p g j -> p (g j)")
                    # global indices+1 of all 64 candidates
                    gidxf = consts.tile([P, NG * 8], F32)
                    nc.vector.tensor_copy(
                        gidxf[:].rearrange("p (g j) -> p g j", g=NG), gidxu[:]
                    )
                    nc.vector.tensor_add(gidxf, gidxf,
                                         gofs[:].rearrange("p g j -> p (g j)"))
                    # top-16 values of the 64 candidates
                    v8a = consts.tile([P, 8], F32)
                    nc.vector.max(v8a, gv)
                    gv2 = consts.tile([P, NG * 8], F32)
                    nc.vector.match_replace(gv2, v8a, gv, -1.0)
                    v8b = consts.tile([P, 8], F32)
                    nc.vector.max(v8b, gv2)
                    ra = consts.tile([P, 1], F32)
                    rb = consts.tile([P, 1], F32)
                    nc.vector.tensor_reduce(ra, v8a, axis=mybir.AxisListType.X,
                                            op=mybir.AluOpType.add)
                    nc.vector.tensor_reduce(rb, v8b, axis=mybir.AxisListType.X,
                                            op=mybir.AluOpType.add)
                    sel_a = consts.tile([P, 1], F32)     # sum of top-16 approx
                    nc.vector.tensor_add(sel_a, ra, rb)
                    # mark the top-16 slots, pull their indices
                    gv3 = consts.tile([P, NG * 8], F32)
                    nc.vector.match_replace(gv3, v8b, gv2, -1.0)
                    mask = consts.tile([P, NG * 8], F32)
                    nc.vector.tensor_scalar(mask, gv3, 0.0, None,
                                            op0=mybir.AluOpType.is_lt)
                    selpos = consts.tile([P, NG * 8], F32)
                    nc.vector.tensor_mul(selpos, mask, gidxf)
                    i8a = consts.tile([P, 8], F32)
                    nc.vector.max(i8a, selpos)
                    sp2 = consts.tile([P, NG * 8], F32)
                    nc.vector.match_replace(sp2, i8a, selpos, 0.0)
                    i8b = consts.tile([P, 8], F32)
                    nc.vector.max(i8b, sp2)
                    # -> gather row ids: (s+1) + (bb*S - 1)
                    idxf = consts.tile([P, NSEL], F32)
                    nc.vector.tensor_scalar_add(idxf[:, :8], i8a, bbofs)
                    nc.vector.tensor_scalar_add(idxf[:, 8:], i8b, bbofs)
                    idxi = consts.tile([P, NSEL], I32)
                    nc.vector.tensor_copy(idxi, idxf)
                    # offsets AP over the 8 used partitions, (q, r, j) order
                    idx_ap = idxi[:].rearrange("(q rest) j -> q rest j", q=4)[
                        :, 0:2, :
                    ]

                    # gather the selected eo columns (one bf16 row per slot)
                    g8 = consts.tile([P, E2], BF16)
                    nc.gpsimd.indirect_dma_start(
                        out=g8[:],
                        out_offset=None,
                        in_=eoG_d[:],
                        in_offset=bass.IndirectOffsetOnAxis(ap=idx_ap, axis=0),
                    )
                    # transpose to contraction-major [e, sel]
                    eoS = consts.tile([P, NEC, P], BF16)
                    for c in range(NEC):
                        pst = tps.tile([P, P], BF16, name=f"pst_{c}")
                        nc.tensor.transpose(
                            pst, g8[:, c * P : (c + 1) * P], ident[:]
                        )
                        nc.vector.tensor_copy(eoS[:, c, :], pst)
                    # exact bf16 projection + tanh + v-dot
                    psE = tps.tile([P, NKC, P], F32)
                    for kc in range(NKC):
                        for c in range(NEC):
                            nc.tensor.matmul(
                                psE[:, kc, :],
                                weTb[:, c, kc * P : (kc + 1) * P],
                                eoS[:, c, :],
                                start=(c == 0), stop=(c == NEC - 1),
                            )
                    enP = consts.tile([P, NKC, P], F32)
                    nc.vector.tensor_add(enP, psE, base_ex)
                    enS = consts.tile([P, NKC, P], BF16)
                    nc.scalar.activation(
                        out=enS, in_=enP,
                        func=mybir.ActivationFunctionType.Tanh,
                    )
                    psD = tps.tile([1, P], F32)
                    for kc in range(NKC):
                        nc.tensor.matmul(
                            psD, vT[:, kc : kc + 1], enS[:, kc, :],
                            start=(kc == 0), stop=(kc == NKC - 1),
                        )
                    expx = consts.tile([1, P], F32)
                    nc.scalar.activation(
                        out=expx, in_=psD,
                        func=mybir.ActivationFunctionType.Exp,
                    )
                    # per-batch sums of the 16 exact exps -> pi(bb) partitions
                    sxs = consts.tile([1, BC], F32)
                    nc.vector.tensor_reduce(
                        sxs, expx[:].rearrange("one (grp j) -> one grp j",
                                               grp=BC),
                        axis=mybir.AxisListType.X, op=mybir.AluOpType.add,
                    )
                    zx_ap = Zx[:].rearrange("(q rest) one -> q rest one",
                                            q=4)[:, 0:2, :]
                    nc.sync.dma_start(out=zx_ap, in_=sxs[:])
                    # corrected denominator + reciprocal
                    Zc = consts.tile([P, 1], F32)
                    nc.vector.tensor_sub(Zc, esum_run, sel_a)
                    nc.vector.tensor_add(Zc, Zc, Zx)
                    rZ = consts.tile([P, 1], F32)
                    nc.vector.reciprocal(rZ, Zc)
                    # 1/Z' along the selection axis
                    rZ8 = consts.tile([1, BC], F32)
                    rz_ap = rZ[:].rearrange("(q rest) one -> q rest one",
                                            q=4)[:, 0:2, :]
                    nc.sync.dma_start(out=rZ8[:], in_=rz_ap)
                    rZ128 = consts.tile([1, P], F32)
                    for bb in range(BC):
                        c = 2 * (bb % 4) + bb // 4
                        cs = _sel_col(bb)
                        nc.vector.tensor_copy(
                            rZ128[:, cs : cs + NSEL],
                            rZ8[:, c : c + 1].to_broadcast([1, NSEL]),
                        )
                    outx = consts.tile([1, P], F32)
                    nc.vector.tensor_mul(outx, expx, rZ128)

                    # normalize the approx output: DVE/ACT split as baseline
                    HALF = 2560
                    nc.vector.tensor_scalar_mul(
                        out_sb[:, :HALF], out_sb[:, :HALF], rZ
                    )
                    nc.scalar.activation(
                        out=out_sb[:, HALF:], in_=out_sb[:, HALF:],
                        func=mybir.ActivationFunctionType.Copy,
                        scale=rZ,
                    )
                    oview = out_d[:].rearrange("(b s) one -> b (s one)", b=BC)
                    osv = out_sb[:].rearrange("(q r) s -> q r s", q=4)
                    d1 = nc.gpsimd.dma_start(out=oview[0:4, :HALF],
                                             in_=osv[:, 0, :HALF])
                    d2 = nc.gpsimd.dma_start(out=oview[4:8, :HALF],
                                             in_=osv[:, 1, :HALF])
                    d3 = nc.gpsimd.dma_start(out=oview[0:4, HALF:],
                                             in_=osv[:, 0, HALF:])
                    d4 = nc.gpsimd.dma_start(out=oview[4:8, HALF:],
                                             in_=osv[:, 1, HALF:])
                    # scatter the exact entries last (same queue + explicit
                    # deps keep it after the bulk writes)
                    sc = nc.gpsimd.indirect_dma_start(
                        out=out_d[:],
                        out_offset=bass.IndirectOffsetOnAxis(ap=idx_ap, axis=0),
                        in_=outx[:],
                        in_offset=None,
                    )
                    for dd in (d1, d2, d3, d4):
                        add_dep_helper(sc.ins, dd.ins,
                                       reason="scatter after bulk output")

    return nc


_nc = None


def _get_nc():
    global _nc
    if _nc is None:
        _nc = build_program()
        _nc.compile()
    return _nc


def _host_prep(hidden, encoder_outputs, W, b, v):
    f8 = ml_dtypes.float8_e4m3
    We = W[:, D:]                                     # [256, 512]
    weT8 = np.ascontiguousarray(
        (SW * We).T.reshape(NEC, P, D).transpose(1, 0, 2)
    ).astype(f8)
    weTb = np.ascontiguousarray(
        We.T.reshape(NEC, P, D).transpose(1, 0, 2)
    ).astype(ml_dtypes.bfloat16)
    Wh = W[:, :D]
    whT = np.ascontiguousarray(
        Wh.reshape(NKC, P, NKC, P).transpose(3, 2, 0, 1)
    )
    bT = np.ascontiguousarray(b.reshape(NKC, P).T)
    vTf = np.ascontiguousarray(v.reshape(NKC, P).T)   # [p, kc]
    vm = np.zeros((P, NKC, BC, 32), dtype=np.float32)
    for bb in range(BC):
        vm[:, :, bb, bb // 4] = vTf
    vm = vm.astype(ml_dtypes.bfloat16)
    vT = vTf.astype(ml_dtypes.bfloat16)
    h = hidden[0]
    bbofs = np.zeros((P, 1), dtype=np.float32)
    for bb in range(BC):
        bbofs[_p_of(bb), 0] = bb * S - 1

    eo8_full = (SE * encoder_outputs).astype(f8)          # [S, B, E2]
    eo8T = np.ascontiguousarray(eo8_full.transpose(1, 2, 0))   # [B, E2, S]
    eobf = encoder_outputs.astype(ml_dtypes.bfloat16)
    eoG_full = np.ascontiguousarray(eobf.transpose(1, 0, 2))   # [B, S, E2]

    in_maps = []
    for idx in range(NCORES):
        bsl = slice(idx * BC, (idx + 1) * BC)
        hT_i = np.ascontiguousarray(
            h[bsl].T.reshape(NKC, P, BC).transpose(1, 0, 2)
        )
        eoT8_i = np.ascontiguousarray(
            eo8T[bsl]
            .reshape(BC, NEC, P, NG, SG)
            .transpose(0, 2, 3, 1, 4)                 # [bb, p, g, c, j]
        )
        eoG_i = np.ascontiguousarray(
            eoG_full[bsl].reshape(BC * S, E2)
        )
        in_maps.append(
            {"eoT8": eoT8_i, "eoG": eoG_i, "weT8": weT8, "weTb": weTb,
             "whT": whT, "hT": hT_i, "bT": bT, "vm": vm, "vT": vT,
             "bbofs": bbofs}
        )
    return in_maps


def kernel(hidden, encoder_outputs, W, b, v):
    hidden = np.asarray(hidden, dtype=np.float32)
    encoder_outputs = np.ascontiguousarray(encoder_outputs, dtype=np.float32)
    W = np.asarray(W, dtype=np.float32)
    b = np.asarray(b, dtype=np.float32)
    v = np.asarray(v, dtype=np.float32)

    in_maps = _host_prep(hidden, encoder_outputs, W, b, v)
    nc = _get_nc()
    try:
        res = run_bass_kernel_spmd(nc, in_maps, list(range(NCORES)))
    except Exception:
        res = run_bass_kernel_spmd(nc, in_maps, list(range(NCORES)))
    global _last_results
    _last_results = res
    out = np.concatenate(
        [res.results[i]["out"].reshape(BC, S) for i in range(NCORES)], axis=0
    )
    return out


_last_results = None


if __name__ == "__main__":
    rng = np.random.default_rng(0)
    inputs = {
        "hidden": rng.standard_normal((1, B, D), dtype=np.float32),
        "encoder_outputs": rng.standard_normal((S, B, E2), dtype=np.float32),
        "W": (rng.standard_normal((D, E2 + D)) * 0.02).astype(np.float32),
        "b": (rng.standard_normal((D,)) * 0.02).astype(np.float32),
        "v": rng.random((D,), dtype=np.float32),
    }
    out = kernel(**inputs)
    print("out", out.shape, out.dtype, out.sum())


# revision 4
# speedup vs baseline: 1.0385x; 1.0385x over previous
"""Cross-attention scores kernel, v2: fp8 DoubleRow + exact top-16 correction.

Pass 1 (approximate, fp8):
    eo is streamed as fp8e4(32*eo) [1 B/elem => ~47us HBM floor/core], and the
    projection We@eo runs as DoubleRow fp8 matmuls (256-deep contraction per
    instruction) against fp8e4(64*We).  tanh(psum/2048 + base) on ACT -> bf16
    energies -> v-dot matmuls -> exp + running sums (as in the bf16 baseline).
    fp8 quantization gives ~7e-2 max rel err on the softmax output -- too big.

Selection (overlapped on idle DVE):
    After each s-group's exp lands in out_sb, max8 + max_index record the
    group's top-8 exp values and indices (per batch; batch bb lives on
    partition 32*(bb%4)+bb//4).

Correction tail:
    From the 64 candidates per batch pick the top 16 by value (max8 +
    match_replace rounds), extract their s-indices, gather those eo columns
    (bf16 rows of eoG) with one 128-row indirect DMA, recompute their scores
    exactly in bf16, and patch: Z' = Z - sum(exp approx@sel) + sum(exp exact@
    sel); output = approx exp * 1/Z' everywhere, then scatter the <=128 exact
    entries over it (same gpsimd DMA queue + explicit dep => ordered last).
    Simulated end-to-end on the real data: max rel err 6.3e-3 (vs 5.7e-3 for
    the all-bf16 baseline, 2e-2 budget).

Sharding: data-parallel over batch (64 -> 8 per core), weights replicated,
no collectives.
"""

import numpy as np
import ml_dtypes

import concourse.bass as bass
import concourse.bacc as bacc
import concourse.tile as tile
from concourse import mybir
from concourse.bass_utils import run_bass_kernel_spmd
from concourse.tile_rust import add_dep_helper
from concourse.masks import make_identity

dt = mybir.dt

S = 4096          # src_len
B = 64            # global batch
E2 = 512          # 2*enc_hid
D = 256           # dec_hid
NCORES = 8
BC = B // NCORES  # batches per core = 8
P = 128
SG = 512          # s-group size
NG = S // SG      # 8 s-groups
NEC = E2 // P     # 4 e-chunks
NKC = D // P      # 2 k-chunks
HB = 4            # batches per block-half
NB = NG * 2       # 16 blocks (s-group x batch-half)
NSEL = 16         # corrected entries per batch

SE = 32.0         # eo fp8 scale (|32*eo| < 240 = e4m3 max finite)
SW = 64.0         # We fp8 scale
INV_SCALE = 1.0 / (SE * SW)

F32 = dt.float32
BF16 = dt.bfloat16
FP8 = dt.float8e4
I32 = dt.int32
U16 = dt.uint16

DR = mybir.MatmulPerfMode.DoubleRow

# CoreSim can't parse partition-subset / multi-level partition APs in DMAs;
# hardware lowering handles them fine (the bf16 baseline shipped them).
SIM_COMPAT = True


def _p_of(bb):
    """Partition where batch bb's scores land (quadrant q=bb%4, col bb//4)."""
    return 32 * (bb % 4) + bb // 4


def _sel_col(bb):
    """Column base in the 128-wide selection axis for batch bb.

    Selection slot order follows the indirect-DMA offset AP iteration
    (q, r, j) with offset partitions pi(bb)=32q+r, so slot = 32q + 16r + j
    for bb = 4r + q.
    """
    q, r = bb % 4, bb // 4
    return 32 * q + 16 * r


def build_program():
    nc = bacc.Bacc(None, target_bir_lowering=False, debug=False, num_devices=8)

    # eoT8[b, p, i, c, j] = fp8(SE * eo[(b//2)*512+j, bb=(b%2)*4+i, c*128+p])
    # block-granular: one contiguous 8KB/partition stream per block
    eoT8_d = nc.declare_dram_parameter("eoT8", [NB, P, HB, NEC, SG], FP8, isOutput=False)
    # eoG[bb*S + s, e] = bf16 eo[s, bb, e]  (gather rows for the exact pass)
    eoG_d = nc.declare_dram_parameter("eoG", [BC * S, E2], BF16, isOutput=False)
    # weT8[p, c, k] = fp8(SW * We.T[c*128+p, k])
    weT8_d = nc.declare_dram_parameter("weT8", [P, NEC, D], FP8, isOutput=False)
    # weTb[p, c, k] = bf16 We.T[c*128+p, k]   (exact pass)
    weTb_d = nc.declare_dram_parameter("weTb", [P, NEC, D], BF16, isOutput=False)
    # whT[p, dc, kc, j] = W[kc*128+j, dc*128+p]  (Wh part, f32)
    whT_d = nc.declare_dram_parameter("whT", [P, NKC, NKC, P], F32, isOutput=False)
    # hT[p, dc, bb] = h[bb, dc*128+p]
    hT_d = nc.declare_dram_parameter("hT", [P, NKC, BC], F32, isOutput=False)
    # bT[p, kc] = bias[kc*128+p]
    bT_d = nc.declare_dram_parameter("bT", [P, NKC], F32, isOutput=False)
    # vm[p, kc, bb, col] = v[kc*128+p] if col == bb//4 else 0  (bf16)
    vm_d = nc.declare_dram_parameter("vm", [P, NKC, BC, 32], BF16, isOutput=False)
    # vT[p, kc] = v[kc*128+p]  (bf16, exact-pass dot)
    vT_d = nc.declare_dram_parameter("vT", [P, NKC], BF16, isOutput=False)
    # bbofs[pi(bb)] = bb*S - 1 (host-built constant; gpsimd memset can't
    # target partition offsets)
    bbofs_d = nc.declare_dram_parameter("bbofs", [P, 1], F32, isOutput=False)
    # flat output: out[bb*S + s] = softmax scores
    out_d = nc.declare_dram_parameter("out", [BC * S, 1], F32, isOutput=True)

    with tile.TileContext(nc) as tc:
        with tc.tile_pool(name="consts", bufs=1) as consts:
            weT8 = consts.tile([P, NEC, D], FP8)
            nc.sync.dma_start(out=weT8, in_=weT8_d[:])
            whT = consts.tile([P, NKC, NKC, P], F32)
            nc.sync.dma_start(out=whT, in_=whT_d[:])
            hT = consts.tile([P, NKC, BC], F32)
            nc.sync.dma_start(out=hT, in_=hT_d[:])
            bT = consts.tile([P, NKC], F32)
            nc.sync.dma_start(out=bT, in_=bT_d[:])
            vm = consts.tile([P, NKC, BC, 32], BF16)
            nc.sync.dma_start(out=vm, in_=vm_d[:])
            weTb = consts.tile([P, NEC, D], BF16)
            nc.scalar.dma_start(out=weTb, in_=weTb_d[:])
            vT = consts.tile([P, NKC], BF16)
            nc.scalar.dma_start(out=vT, in_=vT_d[:])

            baseT = consts.tile([P, NKC, BC], F32)     # [k128, kc, bb]
            base_ex = consts.tile([P, NKC, P], F32)    # [k128, kc, selcol]
            esums = consts.tile([P, NG], F32)
            esum_run = consts.tile([P, 1], F32)
            nc.gpsimd.memset(esum_run, 0.0)
            out_sb = consts.tile([P, S], F32)          # exp(scores)

            gvals = consts.tile([P, NG, 8], F32)       # per-group top-8 exp
            gidxu = consts.tile([P, NG, 8], U16)       # their group-local idx
            gofs = consts.tile([P, NG, 8], F32)        # +g*512+1 offsets
            for g in range(NG):
                nc.gpsimd.memset(gofs[:, g, :], float(g * SG + 1))
            bbofs = consts.tile([P, 1], F32)           # bb*S - 1 on pi(bb)
            nc.scalar.dma_start(out=bbofs, in_=bbofs_d[:])
            ident = consts.tile([P, P], BF16)
            make_identity(nc, ident)
            # Zx preset so unused partitions stay finite through reciprocal
            Zx = consts.tile([P, 1], F32)
            nc.gpsimd.memset(Zx, 1.0)

            # --- init: baseT[k, bb] = sum_d Wh[k, d] h[bb, d] + bias[k] ---
            with tc.tile_pool(name="initps", bufs=1, space="PSUM") as initps:
                ps_base = initps.tile([P, NKC, BC], F32)
                for kc in range(NKC):
                    for dc in range(NKC):
                        nc.tensor.matmul(
                            ps_base[:, kc, :],
                            whT[:, dc, kc, :],
                            hT[:, dc, :],
                            start=(dc == 0),
                            stop=(dc == NKC - 1),
                        )
                for kc in range(NKC):
                    nc.vector.tensor_scalar_add(
                        baseT[:, kc, :], ps_base[:, kc, :], bT[:, kc : kc + 1]
                    )
            # expand baseT along the selection axis: base_ex[:, kc, col(bb)+j]
            for bb in range(BC):
                cs = _sel_col(bb)
                for kc in range(NKC):
                    nc.vector.tensor_copy(
                        base_ex[:, kc, cs : cs + NSEL],
                        baseT[:, kc, bb : bb + 1].to_broadcast([P, NSEL]),
                    )

            with (
                tc.tile_pool(name="eot", bufs=6) as eot_pool,
                tc.tile_pool(name="en", bufs=16) as en_pool,
                tc.tile_pool(name="pep", bufs=7, space="PSUM") as pep_pool,
                tc.tile_pool(name="psc", bufs=1, space="PSUM") as psc_pool,
            ):
                pend = None
                ps_sc = None

                def emit_dots(g, half, ens):
                    """v-dot matmuls for one block-half; 4 concurrent
                    quadrants; at half==1 exp + accum + candidate max8."""
                    nonlocal ps_sc
                    if half == 0:
                        ps_sc = psc_pool.tile([P, SG], F32, tag="psc")
                    for kc in range(NKC):
                        for i in range(HB):
                            bb = half * HB + i
                            q = bb % 4
                            nc.tensor.matmul(
                                ps_sc[32 * q : 32 * q + 32, :],
                                vm[:, kc, bb, :],
                                ens[(i, kc)],
                                start=(half == 0 and kc == 0),
                                stop=(half == 1 and kc == NKC - 1),
                                tile_position=(0, 32 * q),
                                # the sim's zero-region group check
                                # false-positives on multi-quadrant psum
                                # sharing (its value semantics are fine)
                                skip_group_check=True,
                            )
                    if half == 1:
                        s0 = g * SG
                        nc.scalar.activation(
                            out=out_sb[:, s0 : s0 + SG], in_=ps_sc,
                            func=mybir.ActivationFunctionType.Exp,
                            accum_out=esums[:, g : g + 1],
                        )
                        nc.vector.tensor_scalar_add(
                            esum_run, esum_run, esums[:, g : g + 1]
                        )
                        # group candidates for the correction pass
                        nc.vector.max(gvals[:, g, :], out_sb[:, s0 : s0 + SG])
                        nc.vector.max_index(
                            gidxu[:, g, :], gvals[:, g, :],
                            out_sb[:, s0 : s0 + SG],
                        )

                for b in range(NB):
                    g, half = b // 2, b % 2
                    bt = eot_pool.tile([P, HB, NEC, SG], FP8, tag="eot")
                    if b == 0:
                        # first block: per-batch sub-DMAs so the first
                        # matmuls start as soon as slice 0 lands
                        nc.sync.dma_start(
                            out=bt[:, 0, :2, :], in_=eoT8_d[0, :, 0, :2, :]
                        )
                        nc.sync.dma_start(
                            out=bt[:, 0, 2:, :], in_=eoT8_d[0, :, 0, 2:, :]
                        )
                        for i in range(1, HB):
                            nc.sync.dma_start(
                                out=bt[:, i], in_=eoT8_d[0, :, i]
                            )
                    else:
                        nc.sync.dma_start(out=bt, in_=eoT8_d[b])
                    eots = [bt[:, i] for i in range(HB)]

                    ens = {}
                    if b == 0:
                        # batch-outer so the first matmul only waits on tile 0
                        for i in range(HB):
                            bb = half * HB + i
                            for kc in range(NKC):
                                ps = pep_pool.tile(
                                    [P, SG], F32, tag="pep", name=f"ps0_{kc}_{i}"
                                )
                                for j in range(2):
                                    nc.tensor.matmul(
                                        ps,
                                        weT8[:, 2 * j : 2 * j + 2,
                                             kc * P : (kc + 1) * P],
                                        eots[i][:, 2 * j : 2 * j + 2, :],
                                        start=(j == 0), stop=(j == 1),
                                        perf_mode=DR,
                                    )
                                en = en_pool.tile([P, SG], BF16, tag="en",
                                                  name=f"en0_{kc}_{i}")
                                nc.scalar.activation(
                                    out=en, in_=ps,
                                    func=mybir.ActivationFunctionType.Tanh,
                                    bias=baseT[:, kc, bb : bb + 1],
                                    scale=INV_SCALE,
                                )
                                ens[(i, kc)] = en
                        pend = (g, half, ens)
                        continue
                    for kc in range(NKC):
                        pss = [
                            pep_pool.tile([P, SG], F32, tag="pep",
                                          name=f"ps_{b}_{kc}_{i}")
                            for i in range(HB)
                        ]
                        for j in range(2):
                            lhsT = weT8[:, 2 * j : 2 * j + 2,
                                        kc * P : (kc + 1) * P]
                            for i in range(HB):
                                nc.tensor.matmul(
                                    pss[i], lhsT,
                                    eots[i][:, 2 * j : 2 * j + 2, :],
                                    start=(j == 0), stop=(j == 1),
                                    perf_mode=DR,
                                )
                        for i in range(HB):
                            bb = half * HB + i
                            en = en_pool.tile([P, SG], BF16, tag="en")
                            nc.scalar.activation(
                                out=en, in_=pss[i],
                                func=mybir.ActivationFunctionType.Tanh,
                                bias=baseT[:, kc, bb : bb + 1],
                                scale=INV_SCALE,
                            )
                            ens[(i, kc)] = en
                    if pend is not None:
                        emit_dots(*pend)
                    pend = (g, half, ens)

                emit_dots(*pend)

                # ================= correction tail =================
                with (
                    tc.tile_pool(name="tail", bufs=1) as tail,
                    tc.tile_pool(name="tps", bufs=1, space="PSUM") as tps,
                ):
                    gv = gvals[:].rearrange("p g j -> p (g j)")
                    # global indices+1 of all 64 candidates
                    gidxf = consts.tile([P, NG * 8], F32)
                    nc.vector.tensor_copy(
                        gidxf[:].rearrange("p (g j) -> p g j", g=NG), gidxu[:]
                    )
                    nc.vector.tensor_add(gidxf, gidxf,
                                         gofs[:].rearrange("p g j -> p (g j)"))
                    # top-16 values of the 64 candidates
                    v8a = consts.tile([P, 8], F32)
                    nc.vector.max(v8a, gv)
                    gv2 = consts.tile([P, NG * 8], F32)
                    nc.vector.match_replace(gv2, v8a, gv, -1.0)
                    v8b = consts.tile([P, 8], F32)
                    nc.vector.max(v8b, gv2)
                    ra = consts.tile([P, 1], F32)
                    rb = consts.tile([P, 1], F32)
                    nc.vector.tensor_reduce(ra, v8a, axis=mybir.AxisListType.X,
                                            op=mybir.AluOpType.add)
                    nc.vector.tensor_reduce(rb, v8b, axis=mybir.AxisListType.X,
                                            op=mybir.AluOpType.add)
                    sel_a = consts.tile([P, 1], F32)     # sum of top-16 approx
                    nc.vector.tensor_add(sel_a, ra, rb)
                    # mark the top-16 slots, pull their indices
                    gv3 = consts.tile([P, NG * 8], F32)
                    nc.vector.match_replace(gv3, v8b, gv2, -1.0)
                    mask = consts.tile([P, NG * 8], F32)
                    nc.vector.tensor_scalar(mask, gv3, 0.0, None,
                                            op0=mybir.AluOpType.is_lt)
                    selpos = consts.tile([P, NG * 8], F32)
                    nc.vector.tensor_mul(selpos, mask, gidxf)
                    i8a = consts.tile([P, 8], F32)
                    nc.vector.max(i8a, selpos)
                    sp2 = consts.tile([P, NG * 8], F32)
                    nc.vector.match_replace(sp2, i8a, selpos, 0.0)
                    i8b = consts.tile([P, 8], F32)
                    nc.vector.max(i8b, sp2)
                    # -> gather row ids: (s+1) + (bb*S - 1)
                    idxf = consts.tile([P, NSEL], F32)
                    nc.vector.tensor_scalar_add(idxf[:, :8], i8a, bbofs)
                    nc.vector.tensor_scalar_add(idxf[:, 8:], i8b, bbofs)
                    idxi = consts.tile([P, NSEL], I32)
                    nc.vector.tensor_copy(idxi, idxf)
                    # offsets AP over the 8 used partitions, (q, r, j) order
                    idx_ap = idxi[:].rearrange("(q rest) j -> q rest j", q=4)[
                        :, 0:2, :
                    ]

                    # gather the selected eo columns (one bf16 row per slot)
                    g8 = consts.tile([P, E2], BF16)
                    nc.gpsimd.indirect_dma_start(
                        out=g8[:],
                        out_offset=None,
                        in_=eoG_d[:],
                        in_offset=bass.IndirectOffsetOnAxis(ap=idx_ap, axis=0),
                    )
                    # transpose to contraction-major [e, sel]
                    eoS = consts.tile([P, NEC, P], BF16)
                    for c in range(NEC):
                        pst = tps.tile([P, P], BF16, name=f"pst_{c}")
                        nc.tensor.transpose(
                            pst, g8[:, c * P : (c + 1) * P], ident[:]
                        )
                        nc.vector.tensor_copy(eoS[:, c, :], pst)
                    # exact bf16 projection + tanh + v-dot
                    psE = tps.tile([P, NKC, P], F32)
                    for kc in range(NKC):
                        for c in range(NEC):
                            nc.tensor.matmul(
                                psE[:, kc, :],
                                weTb[:, c, kc * P : (kc + 1) * P],
                                eoS[:, c, :],
                                start=(c == 0), stop=(c == NEC - 1),
                            )
                    enP = consts.tile([P, NKC, P], F32)
                    nc.vector.tensor_add(enP, psE, base_ex)
                    enS = consts.tile([P, NKC, P], BF16)
                    nc.scalar.activation(
                        out=enS, in_=enP,
                        func=mybir.ActivationFunctionType.Tanh,
                    )
                    psD = tps.tile([1, P], F32)
                    for kc in range(NKC):
                        nc.tensor.matmul(
                            psD, vT[:, kc : kc + 1], enS[:, kc, :],
                            start=(kc == 0), stop=(kc == NKC - 1),
                        )
                    expx = consts.tile([1, P], F32)
                    nc.scalar.activation(
                        out=expx, in_=psD,
                        func=mybir.ActivationFunctionType.Exp,
                    )
                    # per-batch sums of the 16 exact exps -> pi(bb) partitions
                    sxs = consts.tile([1, BC], F32)
                    nc.vector.tensor_reduce(
                        sxs, expx[:].rearrange("one (grp j) -> one grp j",
                                               grp=BC),
                        axis=mybir.AxisListType.X, op=mybir.AluOpType.add,
                    )
                    zx_ap = Zx[:].rearrange("(q rest) one -> q rest one",
                                            q=4)[:, 0:2, :]
                    nc.sync.dma_start(out=zx_ap, in_=sxs[:])
                    # corrected denominator + reciprocal
                    Zc = consts.tile([P, 1], F32)
                    nc.vector.tensor_sub(Zc, esum_run, sel_a)
                    nc.vector.tensor_add(Zc, Zc, Zx)
                    rZ = consts.tile([P, 1], F32)
                    nc.vector.reciprocal(rZ, Zc)
                    # 1/Z' along the selection axis
                    rZ8 = consts.tile([1, BC], F32)
                    rz_ap = rZ[:].rearrange("(q rest) one -> q rest one",
                                            q=4)[:, 0:2, :]
                    nc.sync.dma_start(out=rZ8[:], in_=rz_ap)
                    rZ128 = consts.tile([1, P], F32)
                    for bb in range(BC):
                        c = 2 * (bb % 4) + bb // 4
                        cs = _sel_col(bb)
                        nc.vector.tensor_copy(
                            rZ128[:, cs : cs + NSEL],
                            rZ8[:, c : c + 1].to_broadcast([1, NSEL]),
                        )
                    outx = consts.tile([1, P], F32)
                    nc.vector.tensor_mul(outx, expx, rZ128)

                    # normalize the approx output: DVE/ACT split as baseline
                    HALF = 2560
                    nc.vector.tensor_scalar_mul(
                        out_sb[:, :HALF], out_sb[:, :HALF], rZ
                    )
                    nc.scalar.activation(
                        out=out_sb[:, HALF:], in_=out_sb[:, HALF:],
                        func=mybir.ActivationFunctionType.Copy,
                        scale=rZ,
                    )
                    oview = out_d[:].rearrange("(b s) one -> b (s one)", b=BC)
                    osv = out_sb[:].rearrange("(q r) s -> q r s", q=4)
                    d1 = nc.gpsimd.dma_start(out=oview[0:4, :HALF],
                                             in_=osv[:, 0, :HALF])
                    d2 = nc.gpsimd.dma_start(out=oview[4:8, :HALF],
                                             in_=osv[:, 1, :HALF])
                    d3 = nc.gpsimd.dma_start(out=oview[0:4, HALF:],
                                             in_=osv[:, 0, HALF:])
                    d4 = nc.gpsimd.dma_start(out=oview[4:8, HALF:],
                                             in_=osv[:, 1, HALF:])
                    # scatter the exact entries last (same queue + explicit
                    # deps keep it after the bulk writes)
                    sc = nc.gpsimd.indirect_dma_start(
                        out=out_d[:],
                        out_offset=bass.IndirectOffsetOnAxis(ap=idx_ap, axis=0),
                        in_=outx[:],
                        in_offset=None,
                    )
                    for dd in (d1, d2, d3, d4):
                        add_dep_helper(sc.ins, dd.ins,
                                       reason="scatter after bulk output")

    return nc


_nc = None


def _get_nc():
    global _nc
    if _nc is None:
        _nc = build_program()
        _nc.compile()
    return _nc


def _host_prep(hidden, encoder_outputs, W, b, v):
    f8 = ml_dtypes.float8_e4m3
    We = W[:, D:]                                     # [256, 512]
    weT8 = np.ascontiguousarray(
        (SW * We).T.reshape(NEC, P, D).transpose(1, 0, 2)
    ).astype(f8)
    weTb = np.ascontiguousarray(
        We.T.reshape(NEC, P, D).transpose(1, 0, 2)
    ).astype(ml_dtypes.bfloat16)
    Wh = W[:, :D]
    whT = np.ascontiguousarray(
        Wh.reshape(NKC, P, NKC, P).transpose(3, 2, 0, 1)
    )
    bT = np.ascontiguousarray(b.reshape(NKC, P).T)
    vTf = np.ascontiguousarray(v.reshape(NKC, P).T)   # [p, kc]
    vm = np.zeros((P, NKC, BC, 32), dtype=np.float32)
    for bb in range(BC):
        vm[:, :, bb, bb // 4] = vTf
    vm = vm.astype(ml_dtypes.bfloat16)
    vT = vTf.astype(ml_dtypes.bfloat16)
    h = hidden[0]
    bbofs = np.zeros((P, 1), dtype=np.float32)
    for bb in range(BC):
        bbofs[_p_of(bb), 0] = bb * S - 1

    eo8_full = (SE * encoder_outputs).astype(f8)          # [S, B, E2]
    eo8T = np.ascontiguousarray(eo8_full.transpose(1, 2, 0))   # [B, E2, S]
    eobf = encoder_outputs.astype(ml_dtypes.bfloat16)
    eoG_full = np.ascontiguousarray(eobf.transpose(1, 0, 2))   # [B, S, E2]

    in_maps = []
    for idx in range(NCORES):
        bsl = slice(idx * BC, (idx + 1) * BC)
        hT_i = np.ascontiguousarray(
            h[bsl].T.reshape(NKC, P, BC).transpose(1, 0, 2)
        )
        arr = eo8T[bsl].reshape(BC, NEC, P, NG, SG)
        # [bb, c, p, g, j] -> blocks [2g+half, p, i, c, j]
        eoT8_i = np.ascontiguousarray(
            arr.transpose(3, 0, 2, 1, 4)              # [g, bb, p, c, j]
            .reshape(NG, 2, HB, P, NEC, SG)
            .transpose(0, 1, 3, 2, 4, 5)              # [g, half, p, i, c, j]
            .reshape(NB, P, HB, NEC, SG)
        )
        eoG_i = np.ascontiguousarray(
            eoG_full[bsl].reshape(BC * S, E2)
        )
        in_maps.append(
            {"eoT8": eoT8_i, "eoG": eoG_i, "weT8": weT8, "weTb": weTb,
             "whT": whT, "hT": hT_i, "bT": bT, "vm": vm, "vT": vT,
             "bbofs": bbofs}
        )
    return in_maps


def kernel(hidden, encoder_outputs, W, b, v):
    hidden = np.asarray(hidden, dtype=np.float32)
    encoder_outputs = np.ascontiguousarray(encoder_outputs, dtype=np.float32)
    W = np.asarray(W, dtype=np.float32)
    b = np.asarray(b, dtype=np.float32)
    v = np.asarray(v, dtype=np.float32)

    in_maps = _host_prep(hidden, encoder_outputs, W, b, v)
    nc = _get_nc()
    try:
        res = run_bass_kernel_spmd(nc, in_maps, list(range(NCORES)))
    except Exception:
        res = run_bass_kernel_spmd(nc, in_maps, list(range(NCORES)))
    global _last_results
    _last_results = res
    out = np.concatenate(
        [res.results[i]["out"].reshape(BC, S) for i in range(NCORES)], axis=0
    )
    return out


_last_results = None


if __name__ == "__main__":
    rng = np.random.default_rng(0)
    inputs = {
        "hidden": rng.standard_normal((1, B, D), dtype=np.float32),
        "encoder_outputs": rng.standard_normal((S, B, E2), dtype=np.float32),
        "W": (rng.standard_normal((D, E2 + D)) * 0.02).astype(np.float32),
        "b": (rng.standard_normal((D,)) * 0.02).astype(np.float32),
        "v": rng.random((D,), dtype=np.float32),
    }
    out = kernel(**inputs)
    print("out", out.shape, out.dtype, out.sum())
